# revision 6
# baseline (speedup 1.0000x reference)
"""GCNConv (DGL GraphConv norm='both') on 8 Trainium2 NeuronCores.

out = D_dst^-1/2 * A * (D_src^-1/2 * X * W) + b
  X: [100000, 32] f32, edge_index: [2, 1600000] (src, dst), W: [32, 32], b: [32]

Sharding: nodes are range-partitioned across the 8 cores (12500 each); each
core owns the aggregation for dst nodes in its range (graph/data parallel).
The host only buckets/sorts/remaps integer indices (plus pure layout ops:
transpose/pad); all floating-point math runs on device.

Device algorithm (single SPMD program):
  Phase 1: core k computes m = (x_k @ W) * outdeg^-1/2 for its own node range
           into COMPACT f16 rows (64B) -> m_own, then AllGather -> m_full
           (6.4 MB, node-major) on every core.
  Phase 2: edges are pre-sorted by (dst window, src&3 lane, local/remote).
           dma_gather needs 256B elements, so each lane's gather table is a
           strided VIEW of the compact table: lane c starts at byte c*64 with
           256B row stride; edge in lane c = src&3 uses index src>>2 (int16-
           safe), so its message is always the first 64B of the element.
           Edges whose src is core-local gather from m_own DURING the
           AllGather (overlap) and are compacted to 64B rows on the idle
           Activation engine; remote edges gather from m_full after. Per dst
           window a one-hot (edge -> local dst) built by one is_equal feeds
           accumulating 128x128x32 matmuls in PSUM. Finalize (indeg^-1/2
           scale + bias + store) runs per span, overlapped.
"""

import os
import sys

import numpy as np

for _p in ("/opt/trn_rl_repo", "/root/.axon_site/_ro/trn_rl_repo"):
    if os.path.isdir(_p) and _p not in sys.path:
        sys.path.insert(0, _p)

N_NODES = 100000
N_CORES = 8
NPC = N_NODES // N_CORES  # 12500 nodes per core
DIN = 32
DOUT = 32
P = 128  # partitions
MROW = 128  # f16 elements per gathered element (256 bytes = 4 node rows)
NTILE = (NPC + P - 1) // P  # 98 dst windows per core
NPAD = NTILE * P
NG = 4  # src lanes (src & 3)
QROWS = N_NODES // NG   # 25000: remote gather-index range per lane
LROWS = NPC // NG       # 3125: local gather-index range per lane
MF_ROWS = N_NODES + 16  # m_full rows incl. tail pad for 256B element overrun
MO_ROWS = NPC + 16      # m_own rows incl. tail pad

SPAN_W = 6  # windows per remote gather span


def _build_program(key):
    """Build the SPMD program.

    key = (RLt, RBt): per-(window, lane) local/remote 128-edge block counts,
    as nested tuples [NTILE][NG] (SPMD-uniform: max over cores).
    """
    from concourse import bacc, bass, mybir, tile

    RLt, RBt = key
    f32 = mybir.dt.float32
    f16 = mybir.dt.float16
    i16 = mybir.dt.int16
    i32 = mybir.dt.int32
    Alu = mybir.AluOpType
    Act = mybir.ActivationFunctionType

    # slot bookkeeping (all SPMD-uniform)
    WSL = [sum(RLt[w]) + sum(RBt[w]) for w in range(NTILE)]  # slots per window
    SLOTS = sum(WSL)
    # local lane streams: slot position of (w, g) run within lane-g stream
    lofs = [[0] * NG for _ in range(NTILE)]
    LSL = [0] * NG
    for g in range(NG):
        acc = 0
        for w in range(NTILE):
            lofs[w][g] = acc
            acc += RLt[w][g]
        LSL[g] = acc
    # remote lane streams
    rofs = [[0] * NG for _ in range(NTILE)]
    RSL = [0] * NG
    for g in range(NG):
        acc = 0
        for w in range(NTILE):
            rofs[w][g] = acc
            acc += RBt[w][g]
        RSL[g] = acc
    # dstloc slot offset of (w, g, local/remote) runs: order per window is
    # [g0 loc, g0 rem, g1 loc, g1 rem, ...]
    sofs = []
    acc = 0
    for w in range(NTILE):
        row = []
        for g in range(NG):
            row.append((acc, acc + RLt[w][g]))
            acc += RLt[w][g] + RBt[w][g]
        sofs.append(row)

    nspan = (NTILE + SPAN_W - 1) // SPAN_W
    maxrsl = max(
        sum(RBt[w0 + j][g] for j in range(min(SPAN_W, NTILE - w0)))
        for w0 in range(0, NTILE, SPAN_W)
        for g in range(NG)
    )

    nc = bacc.Bacc(
        "TRN2",
        target_bir_lowering=False,
        debug=False,
        enable_asserts=False,
        num_devices=N_CORES,
    )

    # ---- I/O ----
    xt_in = nc.dram_tensor("xt_in", [DIN, NPAD], f32, kind="ExternalInput")
    w_in = nc.dram_tensor("w_in", [DIN, DOUT], f32, kind="ExternalInput")
    b_rep = nc.dram_tensor("b_rep", [P, DOUT], f32, kind="ExternalInput")
    qidxL = [
        nc.dram_tensor(f"qidxL{g}", [P, max(LSL[g], 1) * 8], i16,
                       kind="ExternalInput")
        for g in range(NG)
    ]
    qidxR = [
        nc.dram_tensor(f"qidxR{g}", [P, max(RSL[g], 1) * 8], i16,
                       kind="ExternalInput")
        for g in range(NG)
    ]
    dstloc = nc.dram_tensor("dstloc", [P, SLOTS], f16, kind="ExternalInput")
    iota_in = nc.dram_tensor("iota_in", [P, P], f16, kind="ExternalInput")
    dA = nc.dram_tensor("dA", [P, NTILE], i32, kind="ExternalInput")
    dB = nc.dram_tensor("dB", [P, NTILE], i32, kind="ExternalInput")
    oA = nc.dram_tensor("oA", [P, NTILE], i32, kind="ExternalInput")
    oB = nc.dram_tensor("oB", [P, NTILE], i32, kind="ExternalInput")
    out_d = nc.dram_tensor("out_d", [NPAD, DOUT], f32, kind="ExternalOutput")

    # ---- internal DRAM ----
    m_own = nc.dram_tensor("m_own", [MO_ROWS * DOUT], f16, kind="Internal")
    m_full = nc.dram_tensor(
        "m_full", [MF_ROWS * DOUT], f16, kind="Internal", addr_space="Shared"
    )

    with tile.TileContext(nc) as tc:
        with (
            tc.tile_pool(name="const", bufs=1) as cpool,
            tc.tile_pool(name="work", bufs=3) as wpool,
            tc.tile_pool(name="gath", bufs=2) as gpool,
            tc.tile_pool(name="lgath", bufs=1) as lpool,
            tc.tile_pool(name="psum", bufs=4, space="PSUM") as ppool,
            tc.tile_pool(name="psum2", bufs=2, space="PSUM") as ppool2,
        ):
            # ---- critical-path constants only (HWDGE is serial: keep the
            # pre-collective DMA count minimal) ----
            w_t = cpool.tile([DIN, DOUT], f32)
            nc.sync.dma_start(out=w_t[:], in_=w_in[:])
            idx_t = {}
            for nm, h in (("oA", oA), ("oB", oB)):
                t = cpool.tile([P, NTILE], i32, tag=nm)
                nc.sync.dma_start(out=t[:], in_=h[:])
                idx_t[nm] = t

            # ---- out-degree norm (node-major l = n*128 + p) ----
            ns_all = cpool.tile([P, NTILE], f32)
            odeg = wpool.tile([P, NTILE], f32, tag="odeg")
            nc.vector.tensor_tensor(
                out=odeg[:], in0=idx_t["oB"][:], in1=idx_t["oA"][:],
                op=Alu.subtract,
            )
            nc.vector.tensor_scalar_max(out=odeg[:], in0=odeg[:], scalar1=1.0)
            osq = wpool.tile([P, NTILE], f32, tag="osq")
            nc.scalar.activation(out=osq[:], in_=odeg[:], func=Act.Sqrt)
            nc.vector.reciprocal(out=ns_all[:], in_=osq[:])

            # ---- phase 1: m = (x @ W) * ns -> compact f16 rows ----
            G4 = 8  # windows per xT span (1024 nodes)
            for n0 in range(0, NTILE, G4):
                ng = min(G4, NTILE - n0)
                xt = wpool.tile([DIN, G4 * P], f32, tag="xt")
                nc.sync.dma_start(
                    out=xt[:, :ng * P],
                    in_=xt_in[:, n0 * P:(n0 + ng) * P],
                )
                hp4 = ppool2.tile([P, G4, DOUT], f32)
                for j in range(ng):
                    nc.tensor.matmul(
                        out=hp4[:, j, :], lhsT=xt[:, j * P:(j + 1) * P],
                        rhs=w_t[:], start=True, stop=True,
                    )
                m_t = wpool.tile([P, G4, DOUT], f16, tag="m_t")
                nc.vector.tensor_tensor(
                    out=m_t[:, :ng, :], in0=hp4[:, :ng, :],
                    in1=ns_all[:, n0:n0 + ng].unsqueeze(2)
                    .to_broadcast([P, ng, DOUT]),
                    op=Alu.mult,
                )
                lo = n0 * P
                hi = min((n0 + ng) * P, NPC)
                full_tiles = (hi - lo) // P
                mo = m_own[0:NPC * DOUT].rearrange("(n c) -> n c", c=DOUT)
                if full_tiles:
                    nc.sync.dma_start(
                        out=mo[lo:lo + full_tiles * P, :].rearrange(
                            "(j p) c -> p j c", p=P
                        ),
                        in_=m_t[:, :full_tiles, :],
                    )
                rem = (hi - lo) - full_tiles * P
                if rem:
                    nc.sync.dma_start(
                        out=mo[lo + full_tiles * P:hi, :],
                        in_=m_t[:rem, full_tiles, :],
                    )

            # ---- AllGather m (compact: 800KB in -> 6.4MB out) ----
            nc.gpsimd.collective_compute(
                "AllGather",
                mybir.AluOpType.bypass,
                replica_groups=[list(range(N_CORES))],
                ins=[m_own[0:NPC * DOUT].rearrange("(n c) -> n c", c=DOUT)],
                outs=[
                    m_full[0:N_NODES * DOUT].rearrange("(n c) -> n c", c=DOUT)
                ],
            )

            # ---- deferred constant loads (overlap the AllGather) ----
            qidxL_t, qidxR_t = [], []
            for g in range(NG):
                t = cpool.tile([P, max(LSL[g], 1) * 8], i16, tag=f"qL{g}")
                nc.sync.dma_start(out=t[:], in_=qidxL[g][:])
                qidxL_t.append(t)
            iota_t = cpool.tile([P, P], f16)
            nc.sync.dma_start(out=iota_t[:], in_=iota_in[:])
            dst_t = cpool.tile([P, SLOTS], f16)
            nc.sync.dma_start(out=dst_t[:], in_=dstloc[:])
            for g in range(NG):
                t = cpool.tile([P, max(RSL[g], 1) * 8], i16, tag=f"qR{g}")
                nc.sync.dma_start(out=t[:], in_=qidxR[g][:])
                qidxR_t.append(t)
            b_t = cpool.tile([P, DOUT], f32)
            nc.sync.dma_start(out=b_t[:], in_=b_rep[:])
            for nm, h in (("dA", dA), ("dB", dB)):
                t = cpool.tile([P, NTILE], i32, tag=nm)
                nc.sync.dma_start(out=t[:], in_=h[:])
                idx_t[nm] = t

            # in-degree norm (needed per-span at finalize)
            nd_all = cpool.tile([P, NTILE], f32)
            ideg = wpool.tile([P, NTILE], f32, tag="ideg")
            nc.vector.tensor_tensor(
                out=ideg[:], in0=idx_t["dB"][:], in1=idx_t["dA"][:],
                op=Alu.subtract,
            )
            nc.vector.tensor_scalar_max(out=ideg[:], in0=ideg[:], scalar1=1.0)
            isq = wpool.tile([P, NTILE], f32, tag="isq")
            nc.scalar.activation(out=isq[:], in_=ideg[:], func=Act.Sqrt)
            nc.vector.reciprocal(out=nd_all[:], in_=isq[:])

            # per-lane strided gather tables (lane c: byte offset c*64,
            # 256B row stride; index fetches 256B starting at the message)
            laneR = [
                m_full[g * DOUT:g * DOUT + QROWS * MROW].rearrange(
                    "(r e) -> r e", e=MROW
                )
                for g in range(NG)
            ]
            laneL = [
                m_own[g * DOUT:g * DOUT + LROWS * MROW].rearrange(
                    "(r e) -> r e", e=MROW
                )
                for g in range(NG)
            ]

            # ---- local gathers (overlap the AllGather), compact to 64B ----
            qlc = cpool.tile([P, max(sum(LSL), 1), DOUT], f16)
            lbase = [0] * NG
            acc = 0
            for g in range(NG):
                lbase[g] = acc
                acc += LSL[g]
            for g in range(NG):
                if LSL[g] == 0:
                    continue
                n_idx = LSL[g] * P
                qt = lpool.tile([P, max(LSL[g], 1), MROW], f16, tag="ql")
                nc.gpsimd.dma_gather(
                    out_ap=qt[:, :LSL[g], :],
                    in_ap=laneL[g],
                    idxs_ap=qidxL_t[g][:, 0:LSL[g] * 8],
                    num_idxs=n_idx,
                    num_idxs_reg=n_idx,
                    elem_size=MROW,
                    single_packet=False,
                )
                # compact 256B -> 64B rows on the Activation engine
                nc.scalar.activation(
                    out=qlc[:, lbase[g]:lbase[g] + LSL[g], :],
                    in_=qt[:, :LSL[g], 0:DOUT],
                    func=Act.Copy,
                )

            # ---- phase 2: remote gathers + windowed one-hot matmuls ----
            out_stage = cpool.tile([P, NTILE, DOUT + 1], f32)

            q_tiles = [None] * nspan
            span_rofs = [None] * nspan  # per-span within-tile slot offsets

            def ensure_span(sp):
                if q_tiles[sp] is not None:
                    return
                w0 = sp * SPAN_W
                nw = min(SPAN_W, NTILE - w0)
                tiles = []
                offs = []
                for g in range(NG):
                    o = [0] * nw
                    a = 0
                    for j in range(nw):
                        o[j] = a
                        a += RBt[w0 + j][g]
                    offs.append(o)
                    if a == 0:
                        tiles.append(None)
                        continue
                    s0 = rofs[w0][g]
                    qt = gpool.tile([P, maxrsl, MROW], f16, tag=f"q{g}")
                    nc.gpsimd.dma_gather(
                        out_ap=qt[:, :a, :],
                        in_ap=laneR[g],
                        idxs_ap=qidxR_t[g][:, s0 * 8:(s0 + a) * 8],
                        num_idxs=a * P,
                        num_idxs_reg=a * P,
                        elem_size=MROW,
                        single_packet=False,
                    )
                    tiles.append(qt)
                q_tiles[sp] = tiles
                span_rofs[sp] = offs

            for w in range(NTILE):
                sp, wo = divmod(w, SPAN_W)
                ensure_span(sp)
                if sp + 1 < nspan and wo == max(0, SPAN_W - 2):
                    ensure_span(sp + 1)  # prefetch next span
                wsl = WSL[w]
                oh = wpool.tile([P, max(WSL), P + 1], f16, tag="onehot")
                s_base = sum(WSL[:w])
                nc.vector.tensor_tensor(
                    out=oh[:, :wsl, 0:P],
                    in0=iota_t[:].unsqueeze(1).to_broadcast([P, wsl, P]),
                    in1=dst_t[:, s_base:s_base + wsl]
                    .unsqueeze(2).to_broadcast([P, wsl, P]),
                    op=Alu.is_equal,
                )
                ps = ppool.tile([P, DOUT], f32)
                k = 0
                slot = 0
                for g in range(NG):
                    for r in range(RLt[w][g]):
                        nc.tensor.matmul(
                            out=ps[:],
                            lhsT=oh[:, slot, 0:P],
                            rhs=qlc[:, lbase[g] + lofs[w][g] + r, :],
                            start=(k == 0), stop=(k == wsl - 1),
                        )
                        k += 1
                        slot += 1
                    qt = q_tiles[sp][g]
                    ro = span_rofs[sp][g][wo]
                    for r in range(RBt[w][g]):
                        nc.tensor.matmul(
                            out=ps[:],
                            lhsT=oh[:, slot, 0:P],
                            rhs=qt[:, ro + r, 0:DOUT],
                            start=(k == 0), stop=(k == wsl - 1),
                        )
                        k += 1
                        slot += 1
                nc.scalar.activation(
                    out=out_stage[:, w:w + 1, 0:DOUT],
                    in_=ps[:].unsqueeze(1),
                    func=Act.Copy,
                )
                if wo == SPAN_W - 1 or w == NTILE - 1:
                    q_tiles[sp] = None  # allow pool slot reuse
                    # finalize this span: indeg^-1/2 scale + bias + store
                    w0 = sp * SPAN_W
                    nw = w - w0 + 1
                    outt = wpool.tile([P, SPAN_W, DOUT + 1], f32, tag="outt")
                    nc.vector.tensor_tensor(
                        out=outt[:, :nw, 0:DOUT],
                        in0=out_stage[:, w0:w0 + nw, 0:DOUT],
                        in1=nd_all[:, w0:w0 + nw].unsqueeze(2)
                        .to_broadcast([P, nw, DOUT]),
                        op=Alu.mult,
                    )
                    nc.vector.tensor_tensor(
                        out=outt[:, :nw, 0:DOUT], in0=outt[:, :nw, 0:DOUT],
                        in1=b_t[:].unsqueeze(1).to_broadcast([P, nw, DOUT]),
                        op=Alu.add,
                    )
                    nc.sync.dma_start(
                        out=out_d[w0 * P:(w0 + nw) * P, :].rearrange(
                            "(j p) c -> p j c", p=P
                        ),
                        in_=outt[:, :nw, 0:DOUT],
                    )

    nc.compile()
    return nc


def _preprocess(x, edge_index, W, b):
    """Host-side sharding: index bucketing/sorting/remapping + layout."""
    src = np.asarray(edge_index[0], dtype=np.int64)
    dst = np.asarray(edge_index[1], dtype=np.int64)
    x = np.asarray(x, dtype=np.float32)
    W = np.asarray(W, dtype=np.float32)
    b = np.asarray(b, dtype=np.float32)

    core_of = dst // NPC
    per_core = []
    cnts_l = np.zeros((N_CORES, NTILE, NG), dtype=np.int64)
    cnts_r = np.zeros((N_CORES, NTILE, NG), dtype=np.int64)
    for k in range(N_CORES):
        sel = core_of == k
        s_k = src[sel]
        d_k = dst[sel] - k * NPC
        win = d_k // P
        grp = s_k & 3
        loc = ((s_k >= k * NPC) & (s_k < (k + 1) * NPC)).astype(np.int64)
        # within a (window, lane): local edges first, then remote
        order = np.lexsort((s_k, 1 - loc, grp, win))
        s_k, d_k = s_k[order], d_k[order]
        win, grp, loc = win[order], grp[order], loc[order]
        wg = (win * NG + grp) * 2 + (1 - loc)
        c = np.bincount(wg, minlength=NTILE * NG * 2).reshape(NTILE, NG, 2)
        cnts_l[k] = c[:, :, 0]
        cnts_r[k] = c[:, :, 1]
        counts = np.bincount(d_k, minlength=NPC)
        indptr = np.zeros(NPC + 1, dtype=np.int64)
        np.cumsum(counts, out=indptr[1:])
        per_core.append((s_k, d_k, win, grp, loc, indptr))

    # SPMD-uniform per-(window, lane) block counts (max over cores)
    RL = np.ceil(cnts_l.max(axis=0) / P).astype(np.int64)  # [NTILE, NG]
    RB = np.ceil(cnts_r.max(axis=0) / P).astype(np.int64)
    RLt = tuple(tuple(int(v) for v in row) for row in RL)
    RBt = tuple(tuple(int(v) for v in row) for row in RB)

    # slot/stream offset tables (mirror _build_program)
    WSL = RL.sum(axis=1) + RB.sum(axis=1)
    SLOTS = int(WSL.sum())
    lofs = np.zeros((NTILE, NG), dtype=np.int64)
    LSL = np.zeros(NG, dtype=np.int64)
    for g in range(NG):
        lofs[:, g] = np.concatenate([[0], np.cumsum(RL[:, g])[:-1]])
        LSL[g] = RL[:, g].sum()
    rofs = np.zeros((NTILE, NG), dtype=np.int64)
    RSL = np.zeros(NG, dtype=np.int64)
    for g in range(NG):
        rofs[:, g] = np.concatenate([[0], np.cumsum(RB[:, g])[:-1]])
        RSL[g] = RB[:, g].sum()
    # dstloc slot offset of (w, g) local runs; remote follows local
    wbase = np.concatenate([[0], np.cumsum(WSL)[:-1]])
    gbase = np.zeros((NTILE, NG), dtype=np.int64)
    for w in range(NTILE):
        a = wbase[w]
        for g in range(NG):
            gbase[w, g] = a
            a += RL[w, g] + RB[w, g]

    iota_rep = np.broadcast_to(
        np.arange(P, dtype=np.float16)[None, :], (P, P)
    ).copy()
    b_rep = np.broadcast_to(b[None, :], (P, DOUT)).copy()

    def wrap_idx(flat):
        n = len(flat)
        if n == 0:
            return np.zeros((P, 8), dtype=np.int16)
        qi = flat.astype(np.int16).reshape(n // 16, 16).T
        return np.tile(qi, (8, 1))

    in_maps = []
    for k in range(N_CORES):
        s_k, d_k, win, grp, loc, indptr = per_core[k]
        e_srcL = [np.zeros(int(LSL[g]) * P, dtype=np.int64) for g in range(NG)]
        e_srcR = [np.zeros(int(RSL[g]) * P, dtype=np.int64) for g in range(NG)]
        e_dst = np.full(SLOTS * P, P, dtype=np.float16)  # pad sentinel = 128

        # per-(w, g, loc) run positions
        wgl = (win * NG + grp) * 2 + (1 - loc)
        c = np.bincount(wgl, minlength=NTILE * NG * 2)
        starts = np.concatenate([[0], np.cumsum(c)])[:-1]
        pos = np.arange(len(s_k)) - starts[wgl]

        is_l = loc == 1
        # local gather stream: lane-g slot lofs[w,g]+*, idx (src-k*NPC)>>2
        jl = (lofs[win[is_l], grp[is_l]]) * P + pos[is_l]
        for g in range(NG):
            gsel = grp[is_l] == g
            e_srcL[g][jl[gsel]] = (s_k[is_l][gsel] - k * NPC) >> 2
        # remote gather stream: lane-g slot rofs[w,g]+*, idx src>>2
        jr = (rofs[win[~is_l], grp[~is_l]]) * P + pos[~is_l]
        for g in range(NG):
            gsel = grp[~is_l] == g
            e_srcR[g][jr[gsel]] = s_k[~is_l][gsel] >> 2
        # one-hot slot positions: local at gbase, remote after local run
        js = np.where(
            is_l,
            (gbase[win, grp] + pos) * P,
            (gbase[win, grp] + RL[win, grp] + pos) * P,
        ) + 0
        # pos counts within (w,g,loc) run; local slots offset by block pos
        js = np.where(
            is_l,
            (gbase[win, grp]) * P + pos,
            (gbase[win, grp] + RL[win, grp]) * P + pos,
        )
        e_dst[js] = (d_k - win * P).astype(np.float16)

        qarrs = {}
        for g in range(NG):
            qarrs[f"qidxL{g}"] = wrap_idx(e_srcL[g])
            qarrs[f"qidxR{g}"] = wrap_idx(e_srcR[g])
        dstloc_arr = e_dst.reshape(SLOTS, P).T.copy()

        l_idx = np.arange(NPAD)
        valid = l_idx < NPC
        da = np.where(valid, indptr[np.minimum(l_idx, NPC - 1)], 0)
        db = np.where(valid, indptr[np.minimum(l_idx + 1, NPC)], 0)
        dA_ = da.astype(np.int32).reshape(NTILE, P).T.copy()
        dB_ = db.astype(np.int32).reshape(NTILE, P).T.copy()

        lo, hi = k * NPC, (k + 1) * NPC
        sel2 = (src >= lo) & (src < hi)
        ocounts = np.bincount(src[sel2] - lo, minlength=NPC)
        optr = np.zeros(NPC + 1, dtype=np.int64)
        np.cumsum(ocounts, out=optr[1:])
        oa = np.where(valid, optr[np.minimum(l_idx, NPC - 1)], 0)
        ob = np.where(valid, optr[np.minimum(l_idx + 1, NPC)], 0)
        oA_ = oa.astype(np.int32).reshape(NTILE, P).T.copy()
        oB_ = ob.astype(np.int32).reshape(NTILE, P).T.copy()

        xt_k = np.zeros((DIN, NPAD), dtype=np.float32)
        xt_k[:, :NPC] = np.ascontiguousarray(x[lo:hi].T)

        in_maps.append({
            "xt_in": xt_k, "w_in": W, "b_rep": b_rep,
            **qarrs,
            "dstloc": dstloc_arr, "iota_in": iota_rep,
            "dA": dA_, "dB": dB_, "oA": oA_, "oB": oB_,
        })

    return in_maps, (RLt, RBt)


_prog_cache = {}
_last_results = None


def kernel(x, edge_index, W, b):
    from concourse import bass_utils

    in_maps, key = _preprocess(x, edge_index, W, b)
    if key not in _prog_cache:
        _prog_cache[key] = _build_program(key)
    nc = _prog_cache[key]

    res = bass_utils.run_bass_kernel_spmd(
        nc, in_maps, core_ids=list(range(N_CORES))
    )
    global _last_results
    _last_results = res
    outs = []
    for k in range(N_CORES):
        o = res.results[k]["out_d"]  # [NPAD, DOUT], node l = w*128 + p
        outs.append(o[:NPC])
    return np.concatenate(outs, axis=0).astype(np.float32)


# revision 8
# speedup vs baseline: 1.0909x; 1.0909x over previous
"""GCNConv (DGL GraphConv norm='both') on 8 Trainium2 NeuronCores.

out = D_dst^-1/2 * A * (D_src^-1/2 * X * W) + b
  X: [100000, 32] f32, edge_index: [2, 1600000] (src, dst), W: [32, 32], b: [32]

Sharding: nodes are range-partitioned across the 8 cores (12500 each); each
core owns the aggregation for dst nodes in its range (graph/data parallel).
The host only buckets/sorts/remaps integer indices (plus pure layout ops:
transpose/pad); all floating-point math runs on device.

Device algorithm (single SPMD program):
  Phase 1: core k computes m = (x_k @ W) * outdeg^-1/2 for its own node range
           into COMPACT f16 rows (64B) -> m_own, then AllGather -> m_full
           (6.4 MB, node-major) on every core.
  Phase 2: edges are pre-sorted by (dst window, src&3 lane, local/remote).
           dma_gather needs 256B elements, so each lane's gather table is a
           strided VIEW of the compact table: lane c starts at byte c*64 with
           256B row stride; edge in lane c = src&3 uses index src>>2 (int16-
           safe), so its message is always the first 64B of the element.
           Edges whose src is core-local gather from m_own DURING the
           AllGather (overlap) and are compacted to 64B rows on the idle
           Activation engine; remote edges gather from m_full after. Per dst
           window a one-hot (edge -> local dst) built by one is_equal feeds
           accumulating 128x128x32 matmuls in PSUM. Finalize (indeg^-1/2
           scale + bias + store) runs per span, overlapped.
"""

import os
import sys

import numpy as np

for _p in ("/opt/trn_rl_repo", "/root/.axon_site/_ro/trn_rl_repo"):
    if os.path.isdir(_p) and _p not in sys.path:
        sys.path.insert(0, _p)

N_NODES = 100000
N_CORES = 8
NPC = N_NODES // N_CORES  # 12500 nodes per core
DIN = 32
DOUT = 32
P = 128  # partitions
MROW = 128  # f16 elements per gathered element (256 bytes = 4 node rows)
NTILE = (NPC + P - 1) // P  # 98 dst windows per core
NPAD = NTILE * P
NG = 4  # src lanes (src & 3)
QROWS = N_NODES // NG   # 25000: remote gather-index range per lane
LROWS = NPC // NG       # 3125: local gather-index range per lane
MF_ROWS = N_NODES + 16  # m_full rows incl. tail pad for 256B element overrun
MO_ROWS = NPC + 16      # m_own rows incl. tail pad

SPAN_W = 6  # windows per remote gather span


def _build_program(key):
    """Build the SPMD program.

    key = (RLt, RBt): per-(window, lane) local/remote 128-edge block counts,
    as nested tuples [NTILE][NG] (SPMD-uniform: max over cores).
    """
    from concourse import bacc, bass, mybir, tile

    RLt, RBt = key
    f32 = mybir.dt.float32
    f16 = mybir.dt.float16
    i16 = mybir.dt.int16
    i32 = mybir.dt.int32
    Alu = mybir.AluOpType
    Act = mybir.ActivationFunctionType

    # slot bookkeeping (all SPMD-uniform)
    WSL = [sum(RLt[w]) + sum(RBt[w]) for w in range(NTILE)]  # slots per window
    SLOTS = sum(WSL)
    # local lane streams: slot position of (w, g) run within lane-g stream
    lofs = [[0] * NG for _ in range(NTILE)]
    LSL = [0] * NG
    for g in range(NG):
        acc = 0
        for w in range(NTILE):
            lofs[w][g] = acc
            acc += RLt[w][g]
        LSL[g] = acc
    # remote lane streams
    rofs = [[0] * NG for _ in range(NTILE)]
    RSL = [0] * NG
    for g in range(NG):
        acc = 0
        for w in range(NTILE):
            rofs[w][g] = acc
            acc += RBt[w][g]
        RSL[g] = acc
    # dstloc slot offset of (w, g, local/remote) runs: order per window is
    # [g0 loc, g0 rem, g1 loc, g1 rem, ...]
    sofs = []
    acc = 0
    for w in range(NTILE):
        row = []
        for g in range(NG):
            row.append((acc, acc + RLt[w][g]))
            acc += RLt[w][g] + RBt[w][g]
        sofs.append(row)

    nspan = (NTILE + SPAN_W - 1) // SPAN_W
    maxrsl = max(
        sum(RBt[w0 + j][g] for j in range(min(SPAN_W, NTILE - w0)))
        for w0 in range(0, NTILE, SPAN_W)
        for g in range(NG)
    )

    nc = bacc.Bacc(
        "TRN2",
        target_bir_lowering=False,
        debug=False,
        enable_asserts=False,
        num_devices=N_CORES,
    )

    # ---- I/O ----
    xt_in = nc.dram_tensor("xt_in", [DIN, NPAD], f32, kind="ExternalInput")
    w_in = nc.dram_tensor("w_in", [DIN, DOUT], f32, kind="ExternalInput")
    b_rep = nc.dram_tensor("b_rep", [P, DOUT], f32, kind="ExternalInput")
    qidxL = [
        nc.dram_tensor(f"qidxL{g}", [P, max(LSL[g], 1) * 8], i16,
                       kind="ExternalInput")
        for g in range(NG)
    ]
    qidxR = [
        nc.dram_tensor(f"qidxR{g}", [P, max(RSL[g], 1) * 8], i16,
                       kind="ExternalInput")
        for g in range(NG)
    ]
    dstloc = nc.dram_tensor("dstloc", [P, SLOTS], f16, kind="ExternalInput")
    iota_in = nc.dram_tensor("iota_in", [P, P], f16, kind="ExternalInput")
    dA = nc.dram_tensor("dA", [P, NTILE], i32, kind="ExternalInput")
    dB = nc.dram_tensor("dB", [P, NTILE], i32, kind="ExternalInput")
    oA = nc.dram_tensor("oA", [P, NTILE], i32, kind="ExternalInput")
    oB = nc.dram_tensor("oB", [P, NTILE], i32, kind="ExternalInput")
    out_d = nc.dram_tensor("out_d", [NPAD, DOUT], f32, kind="ExternalOutput")

    # ---- internal DRAM ----
    m_own = nc.dram_tensor("m_own", [MO_ROWS * DOUT], f16, kind="Internal")
    m_full = nc.dram_tensor(
        "m_full", [MF_ROWS * DOUT], f16, kind="Internal", addr_space="Shared"
    )

    with tile.TileContext(nc) as tc:
        with (
            tc.tile_pool(name="const", bufs=1) as cpool,
            tc.tile_pool(name="work", bufs=3) as wpool,
            tc.tile_pool(name="gath", bufs=2) as gpool,
            tc.tile_pool(name="lgath", bufs=2) as lpool,
            tc.tile_pool(name="psum", bufs=4, space="PSUM") as ppool,
            tc.tile_pool(name="psum2", bufs=2, space="PSUM") as ppool2,
        ):
            # ---- critical-path constants only (HWDGE is serial: keep the
            # pre-collective DMA count minimal) ----
            w_t = cpool.tile([DIN, DOUT], f32)
            nc.sync.dma_start(out=w_t[:], in_=w_in[:])
            idx_t = {}
            for nm, h in (("oA", oA), ("oB", oB)):
                t = cpool.tile([P, NTILE], i32, tag=nm)
                nc.sync.dma_start(out=t[:], in_=h[:])
                idx_t[nm] = t

            # ---- out-degree norm (node-major l = n*128 + p) ----
            ns_all = cpool.tile([P, NTILE], f32)
            odeg = wpool.tile([P, NTILE], f32, tag="odeg")
            nc.vector.tensor_tensor(
                out=odeg[:], in0=idx_t["oB"][:], in1=idx_t["oA"][:],
                op=Alu.subtract,
            )
            nc.vector.tensor_scalar_max(out=odeg[:], in0=odeg[:], scalar1=1.0)
            osq = wpool.tile([P, NTILE], f32, tag="osq")
            nc.scalar.activation(out=osq[:], in_=odeg[:], func=Act.Sqrt)
            nc.vector.reciprocal(out=ns_all[:], in_=osq[:])

            # ---- phase 1: m = (x @ W) * ns -> compact f16 rows ----
            G4 = 4  # windows per xT span (512 nodes)
            for n0 in range(0, NTILE, G4):
                ng = min(G4, NTILE - n0)
                xt = wpool.tile([DIN, G4 * P], f32, tag="xt")
                nc.sync.dma_start(
                    out=xt[:, :ng * P],
                    in_=xt_in[:, n0 * P:(n0 + ng) * P],
                )
                hp4 = ppool2.tile([P, G4, DOUT], f32)
                for j in range(ng):
                    nc.tensor.matmul(
                        out=hp4[:, j, :], lhsT=xt[:, j * P:(j + 1) * P],
                        rhs=w_t[:], start=True, stop=True,
                    )
                m_t = wpool.tile([P, G4, DOUT], f16, tag="m_t")
                nc.vector.tensor_tensor(
                    out=m_t[:, :ng, :], in0=hp4[:, :ng, :],
                    in1=ns_all[:, n0:n0 + ng].unsqueeze(2)
                    .to_broadcast([P, ng, DOUT]),
                    op=Alu.mult,
                )
                lo = n0 * P
                hi = min((n0 + ng) * P, NPC)
                full_tiles = (hi - lo) // P
                mo = m_own[0:NPC * DOUT].rearrange("(n c) -> n c", c=DOUT)
                if full_tiles:
                    nc.sync.dma_start(
                        out=mo[lo:lo + full_tiles * P, :].rearrange(
                            "(j p) c -> p j c", p=P
                        ),
                        in_=m_t[:, :full_tiles, :],
                    )
                rem = (hi - lo) - full_tiles * P
                if rem:
                    nc.sync.dma_start(
                        out=mo[lo + full_tiles * P:hi, :],
                        in_=m_t[:rem, full_tiles, :],
                    )

            # ---- AllGather m (compact: 800KB in -> 6.4MB out) ----
            nc.gpsimd.collective_compute(
                "AllGather",
                mybir.AluOpType.bypass,
                replica_groups=[list(range(N_CORES))],
                ins=[m_own[0:NPC * DOUT].rearrange("(n c) -> n c", c=DOUT)],
                outs=[
                    m_full[0:N_NODES * DOUT].rearrange("(n c) -> n c", c=DOUT)
                ],
            )

            # ---- deferred constant loads (overlap the AllGather) ----
            qidxL_t, qidxR_t = [], []
            for g in range(NG):
                t = cpool.tile([P, max(LSL[g], 1) * 8], i16, tag=f"qL{g}")
                nc.sync.dma_start(out=t[:], in_=qidxL[g][:])
                qidxL_t.append(t)
            iota_t = cpool.tile([P, P], f16)
            nc.sync.dma_start(out=iota_t[:], in_=iota_in[:])
            dst_t = cpool.tile([P, SLOTS], f16)
            nc.sync.dma_start(out=dst_t[:], in_=dstloc[:])
            for g in range(NG):
                t = cpool.tile([P, max(RSL[g], 1) * 8], i16, tag=f"qR{g}")
                nc.sync.dma_start(out=t[:], in_=qidxR[g][:])
                qidxR_t.append(t)
            b_t = cpool.tile([P, DOUT], f32)
            nc.sync.dma_start(out=b_t[:], in_=b_rep[:])
            for nm, h in (("dA", dA), ("dB", dB)):
                t = cpool.tile([P, NTILE], i32, tag=nm)
                nc.sync.dma_start(out=t[:], in_=h[:])
                idx_t[nm] = t

            # in-degree norm (needed per-span at finalize)
            nd_all = cpool.tile([P, NTILE], f32)
            ideg = wpool.tile([P, NTILE], f32, tag="ideg")
            nc.vector.tensor_tensor(
                out=ideg[:], in0=idx_t["dB"][:], in1=idx_t["dA"][:],
                op=Alu.subtract,
            )
            nc.vector.tensor_scalar_max(out=ideg[:], in0=ideg[:], scalar1=1.0)
            isq = wpool.tile([P, NTILE], f32, tag="isq")
            nc.scalar.activation(out=isq[:], in_=ideg[:], func=Act.Sqrt)
            nc.vector.reciprocal(out=nd_all[:], in_=isq[:])

            # per-lane strided gather tables (lane c: byte offset c*64,
            # 256B row stride; index fetches 256B starting at the message)
            laneR = [
                m_full[g * DOUT:g * DOUT + QROWS * MROW].rearrange(
                    "(r e) -> r e", e=MROW
                )
                for g in range(NG)
            ]
            laneL = [
                m_own[g * DOUT:g * DOUT + LROWS * MROW].rearrange(
                    "(r e) -> r e", e=MROW
                )
                for g in range(NG)
            ]

            # ---- local gathers (overlap the AllGather), compact to 64B ----
            qlc = cpool.tile([P, max(sum(LSL), 1), DOUT], f16)
            lbase = [0] * NG
            acc = 0
            for g in range(NG):
                lbase[g] = acc
                acc += LSL[g]
            for g in range(NG):
                if LSL[g] == 0:
                    continue
                n_idx = LSL[g] * P
                qt = lpool.tile([P, max(LSL[g], 1), MROW], f16, tag="ql")
                nc.gpsimd.dma_gather(
                    out_ap=qt[:, :LSL[g], :],
                    in_ap=laneL[g],
                    idxs_ap=qidxL_t[g][:, 0:LSL[g] * 8],
                    num_idxs=n_idx,
                    num_idxs_reg=n_idx,
                    elem_size=MROW,
                    single_packet=False,
                )
                # compact 256B -> 64B rows on the Activation engine
                nc.scalar.activation(
                    out=qlc[:, lbase[g]:lbase[g] + LSL[g], :],
                    in_=qt[:, :LSL[g], 0:DOUT],
                    func=Act.Copy,
                )

            # ---- phase 2: remote gathers + windowed one-hot matmuls ----
            out_stage = cpool.tile([P, NTILE, DOUT + 1], f32)

            q_tiles = [None] * nspan
            span_rofs = [None] * nspan  # per-span within-tile slot offsets

            def ensure_span(sp):
                if q_tiles[sp] is not None:
                    return
                w0 = sp * SPAN_W
                nw = min(SPAN_W, NTILE - w0)
                tiles = []
                offs = []
                for g in range(NG):
                    o = [0] * nw
                    a = 0
                    for j in range(nw):
                        o[j] = a
                        a += RBt[w0 + j][g]
                    offs.append(o)
                    if a == 0:
                        tiles.append(None)
                        continue
                    s0 = rofs[w0][g]
                    qt = gpool.tile([P, maxrsl, MROW], f16, tag=f"q{g}")
                    nc.gpsimd.dma_gather(
                        out_ap=qt[:, :a, :],
                        in_ap=laneR[g],
                        idxs_ap=qidxR_t[g][:, s0 * 8:(s0 + a) * 8],
                        num_idxs=a * P,
                        num_idxs_reg=a * P,
                        elem_size=MROW,
                        single_packet=False,
                    )
                    tiles.append(qt)
                q_tiles[sp] = tiles
                span_rofs[sp] = offs

            for w in range(NTILE):
                sp, wo = divmod(w, SPAN_W)
                ensure_span(sp)
                if sp + 1 < nspan and wo == max(0, SPAN_W - 2):
                    ensure_span(sp + 1)  # prefetch next span
                wsl = WSL[w]
                oh = wpool.tile([P, max(WSL), P + 1], f16, tag="onehot")
                s_base = sum(WSL[:w])
                nc.vector.tensor_tensor(
                    out=oh[:, :wsl, 0:P],
                    in0=iota_t[:].unsqueeze(1).to_broadcast([P, wsl, P]),
                    in1=dst_t[:, s_base:s_base + wsl]
                    .unsqueeze(2).to_broadcast([P, wsl, P]),
                    op=Alu.is_equal,
                )
                ps = ppool.tile([P, DOUT], f32)
                k = 0
                slot = 0
                for g in range(NG):
                    for r in range(RLt[w][g]):
                        nc.tensor.matmul(
                            out=ps[:],
                            lhsT=oh[:, slot, 0:P],
                            rhs=qlc[:, lbase[g] + lofs[w][g] + r, :],
                            start=(k == 0), stop=(k == wsl - 1),
                        )
                        k += 1
                        slot += 1
                    qt = q_tiles[sp][g]
                    ro = span_rofs[sp][g][wo]
                    for r in range(RBt[w][g]):
                        nc.tensor.matmul(
                            out=ps[:],
                            lhsT=oh[:, slot, 0:P],
                            rhs=qt[:, ro + r, 0:DOUT],
                            start=(k == 0), stop=(k == wsl - 1),
                        )
                        k += 1
                        slot += 1
                nc.scalar.activation(
                    out=out_stage[:, w:w + 1, 0:DOUT],
                    in_=ps[:].unsqueeze(1),
                    func=Act.Copy,
                )
                if wo == SPAN_W - 1 or w == NTILE - 1:
                    q_tiles[sp] = None  # allow pool slot reuse
                    # finalize this span: indeg^-1/2 scale + bias + store
                    w0 = sp * SPAN_W
                    nw = w - w0 + 1
                    outt = wpool.tile([P, SPAN_W, DOUT + 1], f32, tag="outt")
                    nc.vector.tensor_tensor(
                        out=outt[:, :nw, 0:DOUT],
                        in0=out_stage[:, w0:w0 + nw, 0:DOUT],
                        in1=nd_all[:, w0:w0 + nw].unsqueeze(2)
                        .to_broadcast([P, nw, DOUT]),
                        op=Alu.mult,
                    )
                    nc.vector.tensor_tensor(
                        out=outt[:, :nw, 0:DOUT], in0=outt[:, :nw, 0:DOUT],
                        in1=b_t[:].unsqueeze(1).to_broadcast([P, nw, DOUT]),
                        op=Alu.add,
                    )
                    nc.sync.dma_start(
                        out=out_d[w0 * P:(w0 + nw) * P, :].rearrange(
                            "(j p) c -> p j c", p=P
                        ),
                        in_=outt[:, :nw, 0:DOUT],
                    )

    nc.compile()
    return nc


def _preprocess(x, edge_index, W, b):
    """Host-side sharding: index bucketing/sorting/remapping + layout."""
    src = np.asarray(edge_index[0], dtype=np.int64)
    dst = np.asarray(edge_index[1], dtype=np.int64)
    x = np.asarray(x, dtype=np.float32)
    W = np.asarray(W, dtype=np.float32)
    b = np.asarray(b, dtype=np.float32)

    core_of = dst // NPC
    per_core = []
    cnts_l = np.zeros((N_CORES, NTILE, NG), dtype=np.int64)
    cnts_r = np.zeros((N_CORES, NTILE, NG), dtype=np.int64)
    for k in range(N_CORES):
        sel = core_of == k
        s_k = src[sel]
        d_k = dst[sel] - k * NPC
        win = d_k // P
        grp = s_k & 3
        loc = ((s_k >= k * NPC) & (s_k < (k + 1) * NPC)).astype(np.int64)
        # within a (window, lane): local edges first, then remote
        order = np.lexsort((s_k, 1 - loc, grp, win))
        s_k, d_k = s_k[order], d_k[order]
        win, grp, loc = win[order], grp[order], loc[order]
        wg = (win * NG + grp) * 2 + (1 - loc)
        c = np.bincount(wg, minlength=NTILE * NG * 2).reshape(NTILE, NG, 2)
        cnts_l[k] = c[:, :, 0]
        cnts_r[k] = c[:, :, 1]
        counts = np.bincount(d_k, minlength=NPC)
        indptr = np.zeros(NPC + 1, dtype=np.int64)
        np.cumsum(counts, out=indptr[1:])
        per_core.append((s_k, d_k, win, grp, loc, indptr))

    # SPMD-uniform per-(window, lane) block counts (max over cores)
    RL = np.ceil(cnts_l.max(axis=0) / P).astype(np.int64)  # [NTILE, NG]
    RB = np.ceil(cnts_r.max(axis=0) / P).astype(np.int64)
    RLt = tuple(tuple(int(v) for v in row) for row in RL)
    RBt = tuple(tuple(int(v) for v in row) for row in RB)

    # slot/stream offset tables (mirror _build_program)
    WSL = RL.sum(axis=1) + RB.sum(axis=1)
    SLOTS = int(WSL.sum())
    lofs = np.zeros((NTILE, NG), dtype=np.int64)
    LSL = np.zeros(NG, dtype=np.int64)
    for g in range(NG):
        lofs[:, g] = np.concatenate([[0], np.cumsum(RL[:, g])[:-1]])
        LSL[g] = RL[:, g].sum()
    rofs = np.zeros((NTILE, NG), dtype=np.int64)
    RSL = np.zeros(NG, dtype=np.int64)
    for g in range(NG):
        rofs[:, g] = np.concatenate([[0], np.cumsum(RB[:, g])[:-1]])
        RSL[g] = RB[:, g].sum()
    # dstloc slot offset of (w, g) local runs; remote follows local
    wbase = np.concatenate([[0], np.cumsum(WSL)[:-1]])
    gbase = np.zeros((NTILE, NG), dtype=np.int64)
    for w in range(NTILE):
        a = wbase[w]
        for g in range(NG):
            gbase[w, g] = a
            a += RL[w, g] + RB[w, g]

    iota_rep = np.broadcast_to(
        np.arange(P, dtype=np.float16)[None, :], (P, P)
    ).copy()
    b_rep = np.broadcast_to(b[None, :], (P, DOUT)).copy()

    def wrap_idx(flat):
        n = len(flat)
        if n == 0:
            return np.zeros((P, 8), dtype=np.int16)
        qi = flat.astype(np.int16).reshape(n // 16, 16).T
        return np.tile(qi, (8, 1))

    in_maps = []
    for k in range(N_CORES):
        s_k, d_k, win, grp, loc, indptr = per_core[k]
        e_srcL = [np.zeros(int(LSL[g]) * P, dtype=np.int64) for g in range(NG)]
        e_srcR = [np.zeros(int(RSL[g]) * P, dtype=np.int64) for g in range(NG)]
        e_dst = np.full(SLOTS * P, P, dtype=np.float16)  # pad sentinel = 128

        # per-(w, g, loc) run positions
        wgl = (win * NG + grp) * 2 + (1 - loc)
        c = np.bincount(wgl, minlength=NTILE * NG * 2)
        starts = np.concatenate([[0], np.cumsum(c)])[:-1]
        pos = np.arange(len(s_k)) - starts[wgl]

        is_l = loc == 1
        # local gather stream: lane-g slot lofs[w,g]+*, idx (src-k*NPC)>>2
        jl = (lofs[win[is_l], grp[is_l]]) * P + pos[is_l]
        for g in range(NG):
            gsel = grp[is_l] == g
            e_srcL[g][jl[gsel]] = (s_k[is_l][gsel] - k * NPC) >> 2
        # remote gather stream: lane-g slot rofs[w,g]+*, idx src>>2
        jr = (rofs[win[~is_l], grp[~is_l]]) * P + pos[~is_l]
        for g in range(NG):
            gsel = grp[~is_l] == g
            e_srcR[g][jr[gsel]] = s_k[~is_l][gsel] >> 2
        # one-hot slot positions: local at gbase, remote after local run
        js = np.where(
            is_l,
            (gbase[win, grp] + pos) * P,
            (gbase[win, grp] + RL[win, grp] + pos) * P,
        ) + 0
        # pos counts within (w,g,loc) run; local slots offset by block pos
        js = np.where(
            is_l,
            (gbase[win, grp]) * P + pos,
            (gbase[win, grp] + RL[win, grp]) * P + pos,
        )
        e_dst[js] = (d_k - win * P).astype(np.float16)

        qarrs = {}
        for g in range(NG):
            qarrs[f"qidxL{g}"] = wrap_idx(e_srcL[g])
            qarrs[f"qidxR{g}"] = wrap_idx(e_srcR[g])
        dstloc_arr = e_dst.reshape(SLOTS, P).T.copy()

        l_idx = np.arange(NPAD)
        valid = l_idx < NPC
        da = np.where(valid, indptr[np.minimum(l_idx, NPC - 1)], 0)
        db = np.where(valid, indptr[np.minimum(l_idx + 1, NPC)], 0)
        dA_ = da.astype(np.int32).reshape(NTILE, P).T.copy()
        dB_ = db.astype(np.int32).reshape(NTILE, P).T.copy()

        lo, hi = k * NPC, (k + 1) * NPC
        sel2 = (src >= lo) & (src < hi)
        ocounts = np.bincount(src[sel2] - lo, minlength=NPC)
        optr = np.zeros(NPC + 1, dtype=np.int64)
        np.cumsum(ocounts, out=optr[1:])
        oa = np.where(valid, optr[np.minimum(l_idx, NPC - 1)], 0)
        ob = np.where(valid, optr[np.minimum(l_idx + 1, NPC)], 0)
        oA_ = oa.astype(np.int32).reshape(NTILE, P).T.copy()
        oB_ = ob.astype(np.int32).reshape(NTILE, P).T.copy()

        xt_k = np.zeros((DIN, NPAD), dtype=np.float32)
        xt_k[:, :NPC] = np.ascontiguousarray(x[lo:hi].T)

        in_maps.append({
            "xt_in": xt_k, "w_in": W, "b_rep": b_rep,
            **qarrs,
            "dstloc": dstloc_arr, "iota_in": iota_rep,
            "dA": dA_, "dB": dB_, "oA": oA_, "oB": oB_,
        })

    return in_maps, (RLt, RBt)


_prog_cache = {}
_last_results = None


def kernel(x, edge_index, W, b):
    from concourse import bass_utils

    in_maps, key = _preprocess(x, edge_index, W, b)
    if key not in _prog_cache:
        _prog_cache[key] = _build_program(key)
    nc = _prog_cache[key]

    res = bass_utils.run_bass_kernel_spmd(
        nc, in_maps, core_ids=list(range(N_CORES))
    )
    global _last_results
    _last_results = res
    outs = []
    for k in range(N_CORES):
        o = res.results[k]["out_d"]  # [NPAD, DOUT], node l = w*128 + p
        outs.append(o[:NPC])
    return np.concatenate(outs, axis=0).astype(np.float32)
